# revision 2
# baseline (speedup 1.0000x reference)
"""Trainium2 Bass kernel for 3D conv-attention layer — cost-model-optimized.

Math folds (exact up to dtype rounding):
  scoresT[j,i] = (q+bq)^T(k+bk) = x^T M x + r^T x + f(j) + const, where
  M = Wq^T Wk, r = Wk^T bq, and the f(j)/const terms cancel in the softmax
  over i. So k and q are never materialized: one fp32r projection
  [M | Wv | r] produces g = Mx, v, and rx; scores are per-site bf16
  matmuls x^T g with the r^T x row accumulated via rank-1 matmuls.
  bk drops exactly; bv commutes through softmax (columns sum to 1) into
  bo' = bo + Wo bv, which rides the output projection's bias row.

Structure per (b,h) chunk ([C=64, 2048] with free = 32*w + d, 64 sites):
  - x DMA'd (float32r bits) into partitions 64-127 of oext =
    [o(0:32); ones(32); zeros(33:64); x(64:128)]; a host-prepared bf16
    copy of x lands in its own [64,2048] tile (scores LHS).
  - proj: [M|Wv|r] fp32r (N=512, 1cyc/row) -> g@p0-63, v@p64-95, rx@p96,
    drained in 4 [97,512] ACT ops to bf16.
  - scores per site (bf16, accumulating rank-1 bq term); softmax over
    free i: exp on ACT, den/rcp/scale/block-transpose on DVE in two
    256-col half-chains overlapped with the second score pass.
  - vT via PE is_transpose quad transposes into a bf16 view of one bank.
  - o per site (bf16) -> drained as fp32r into oext[0:32].
  - output projection LHS [WoT; bo'; 0; I64] fp32r: y = Wo o + bo' + x
    in one matmul; y drained to bf16 and DMA'd (host upcasts).
Host pre-transposes x to [B, HS, C, W*D]; inverse-transposes y.
"""

import math
from contextlib import ExitStack

import numpy as np
import ml_dtypes

import concourse.bass as bass
import concourse.mybir as mybir
from concourse import bacc
import concourse.tile as tile
from concourse.bass_utils import run_bass_kernel_spmd

B, C, D, H, W = 4, 64, 32, 64, 64
S = C // 2  # 32
NCORES = 8
HS = H // NCORES
NCH = B * HS  # chunks per core
F32 = mybir.dt.float32
BF16 = mybir.dt.bfloat16
FR = mybir.dt.float32r

INV_SQRT_S = 1.0 / math.sqrt(S)
ACTF = mybir.ActivationFunctionType


def mkap(base, part0, pcount, foff, fdims):
    full = base[...] if not isinstance(base, bass.AP) else base
    pstride = full.ap[0][0]
    return bass.AP(tensor=full.tensor,
                   offset=full.offset + part0 * pstride + foff,
                   ap=[[pstride, pcount]] + [list(d) for d in fdims])


def build_program():
    nc = bacc.Bacc()
    x_d = nc.declare_dram_parameter("x", [B, HS, C, W * D], FR, isOutput=False)
    xb_d = nc.declare_dram_parameter("xb", [B, HS, C, W * D], BF16,
                                     isOutput=False)
    wvg_d = nc.declare_dram_parameter("wvg", [128, 128], F32, isOutput=False)
    woe_d = nc.declare_dram_parameter("woe", [128, 64], F32, isOutput=False)
    idm_d = nc.declare_dram_parameter("idm", [128, 32], F32, isOutput=False)
    oz_d = nc.declare_dram_parameter("oz", [32, W * D], FR, isOutput=False)
    y_d = nc.declare_dram_parameter("y", [B, HS, C, W * D], BF16, isOutput=True)

    FD = W * D  # 2048

    with tile.TileContext(nc) as tc, ExitStack() as ctx:
        const = ctx.enter_context(tc.tile_pool(name="const", bufs=1))
        sb = ctx.enter_context(tc.tile_pool(name="sb", bufs=3))
        ps = ctx.enter_context(tc.tile_pool(name="ps", bufs=1, space="PSUM"))

        # ---- constants ----
        wvg_s = const.tile([128, 128], F32, tag="wvgs")
        wvg = const.tile([128, 128], FR, tag="wvg")
        woe_s = const.tile([128, 64], F32, tag="woes")
        woe = const.tile([128, 64], FR, tag="woe")
        idm_s = const.tile([128, 32], F32, tag="idms")
        idm = const.tile([128, 32], BF16, tag="idm")
        for dst_s, dst, src in ((wvg_s, wvg, wvg_d), (woe_s, woe, woe_d),
                                (idm_s, idm, idm_d)):
            nc.sync.dma_start(out=dst_s[:, :], in_=src[:, :])
            nc.vector.tensor_copy(out=dst[:, :], in_=dst_s[:, :])

        # oext: [o(0:32); ones/zeros(32:64); x(64:128)], float32r, 3 parities
        oep = ctx.enter_context(tc.tile_pool(name="oep", bufs=1))
        oext = []
        xbt = []
        for par in range(3):
            t = oep.tile([128, FD], FR, tag=f"oe{par}", name=f"oext{par}")
            nc.sync.dma_start(out=t[32:64, :], in_=oz_d[:, :])
            oext.append(t)
            xt = oep.tile([65, FD], BF16, tag=f"xb{par}", name=f"xbt{par}")
            nc.vector.memset(xt[64:65, :], 1.0)
            xbt.append(xt)

        def chunk_bh(cidx):
            return cidx // HS, cidx % HS

        def emit_xdma(cidx):
            b, h = chunk_bh(cidx)
            nc.sync.dma_start(out=oext[cidx % 3][64:128, :],
                              in_=x_d[b, h, :, :])

        def emit_xbdma(cidx):
            b, h = chunk_bh(cidx)
            xb_sb = xbt[cidx % 3]
            nc.sync.dma_start(out=xb_sb[0:64, :], in_=xb_d[b, h, :, :])
            return xb_sb

        def emit_proj(cidx):
            """[M|Wv|r] projection + vg drains. Returns vg_sb."""
            vg_sb = sb.tile([128, FD], BF16, tag="vg")
            x_fr = oext[cidx % 3]
            for i in range(4):
                pt = ps.tile([128, 512], F32, tag=f"P{i % 2}", name=f"pj{i}")
                nc.tensor.matmul(pt[0:128, :], wvg[64:128, :],
                                 x_fr[64:128, 512 * i:512 * i + 512],
                                 start=True, stop=True)
                nc.scalar.activation(vg_sb[0:128, 512 * i:512 * i + 512],
                                     pt[0:128, :], ACTF.Copy)
            return vg_sb

        def emit_attention(cidx, vg_sb, xb_sb):
            """scores + vt transposes + softmax; returns (a_sb, vt_sb).

            S-bank 0 (even quads) scores run first so its softmax half-chain
            overlaps S-bank 1's scores. All vt transposes share array tile
            (64,0) and serialize on weight loads -> one bf16-view bank.
            """
            scb = ps.tile([128, 512], F32, tag="S0", name="scb")
            vtp = ps.tile([128, 512], F32, tag="V0", name="vtp").bitcast(BF16)
            e_sb = sb.tile([128, 512], BF16, tag="e")
            den = sb.tile([128, 16], BF16, tag="den")
            rcp = sb.tile([128, 16], BF16, tag="rcp")
            es = sb.tile([128, 512], BF16, tag="es")
            a_sb = sb.tile([128, 512], BF16, tag="a")
            vt_sb = sb.tile([128, 512], BF16, tag="vt")

            def half_chain(bnk):
                c0 = 256 * bnk
                nc.scalar.activation(e_sb[:, c0:c0 + 256],
                                     scb[:, c0:c0 + 256],
                                     ACTF.Exp, scale=INV_SQRT_S)
                with nc.allow_low_precision(reason="bf16 softmax denom"):
                    nc.vector.reduce_sum(
                        out=den[:, 8 * bnk:8 * bnk + 8],
                        in_=mkap(e_sb, 0, 128, c0, [[32, 8], [1, 32]]),
                        axis=mybir.AxisListType.X)
                    nc.vector.reciprocal(rcp[:, 8 * bnk:8 * bnk + 8],
                                         den[:, 8 * bnk:8 * bnk + 8])
                nc.gpsimd.tensor_tensor(
                    out=mkap(es, 0, 128, c0, [[32, 8], [1, 32]]),
                    in0=mkap(e_sb, 0, 128, c0, [[32, 8], [1, 32]]),
                    in1=mkap(rcp, 0, 128, 8 * bnk, [[1, 8], [0, 32]]),
                    op=mybir.AluOpType.mult)
                nc.vector.transpose(a_sb[:, c0:c0 + 256], es[:, c0:c0 + 256])

            for bnk in range(2):
                for j, qd in enumerate(range(8 * bnk, 8 * bnk + 8)):
                    for u in range(4):
                        w = 4 * qd + u
                        nc.tensor.matmul(
                            scb[32 * u:32 * u + 32, 32 * qd:32 * qd + 32],
                            xb_sb[0:65, 32 * w:32 * w + 32],
                            vg_sb[0:65, 32 * w:32 * w + 32],
                            start=True, stop=True,
                            tile_position=(0, 32 * u))
                    if bnk == 0:
                        for vq in (2 * j, 2 * j + 1):
                            nc.tensor.transpose(
                                vtp[:, 32 * vq:32 * vq + 32],
                                vg_sb[96:128, 128 * vq:128 * vq + 128],
                                idm[96:128, :], tile_position=(96, 0))
                half_chain(bnk)
                if bnk == 0:
                    nc.scalar.activation(vt_sb[:, :], vtp[:, 0:512],
                                         ACTF.Copy)
            return a_sb, vt_sb

        def emit_o_outproj(cidx, a_sb, vt_sb):
            # o-bank u = w%4; the 16 matmuls per bank share one array tile
            # (32u, 0) and serialize on weight reloads; drains are strided.
            b, h = chunk_bh(cidx)
            oe = oext[cidx % 3]
            y_sb = sb.tile([64, FD], BF16, tag="y")
            orot = [ps.tile([128, 512], F32, tag=t, name=f"orot{t}")
                    for t in ("O0", "O1", "S1", "X0")]
            o_eng = (nc.vector, nc.scalar, nc.vector, nc.scalar)
            y_eng = (nc.vector, nc.scalar, nc.vector, nc.scalar)
            y_tags = ("P0", "P1", "O0", "O1")
            for u in range(4):
                for qd in range(16):
                    nc.tensor.matmul(
                        orot[u][0:32, 32 * qd:32 * qd + 32],
                        vt_sb[32 * u:32 * u + 32, 32 * qd:32 * qd + 32],
                        a_sb[32 * u:32 * u + 32, 32 * qd:32 * qd + 32],
                        start=True, stop=True,
                        tile_position=(32 * u, 0))
                # drain bank u in halves: sites w=4m+u -> oext col 128m+32u
                for hf in range(2):
                    srcap = mkap(orot[u], 0, 32, 256 * hf, [[32, 8], [1, 32]])
                    dstap = mkap(oe, 0, 32, 1024 * hf + 32 * u,
                                 [[128, 8], [1, 32]])
                    eng = o_eng[(2 * u + hf) % 4]
                    if eng is nc.scalar:
                        nc.scalar.activation(dstap, srcap, ACTF.Copy)
                    else:
                        eng.tensor_copy(out=dstap, in_=srcap)
            for cq in range(4):
                yt = ps.tile([128, 512], F32, tag=y_tags[cq], name=f"yt{cq}")
                nc.tensor.matmul(yt[0:64, :], woe[:, :],
                                 oe[:, 512 * cq:512 * cq + 512],
                                 start=True, stop=True)
                dst = y_sb[0:64, 512 * cq:512 * cq + 512]
                if y_eng[cq] is nc.scalar:
                    nc.scalar.activation(dst, yt[0:64, :], ACTF.Copy)
                else:
                    y_eng[cq].tensor_copy(out=dst, in_=yt[0:64, :])
            nc.sync.dma_start(out=y_d[b, h, :, :], in_=y_sb[0:64, :])

        # ---- software-pipelined emission (depth 2) ----
        emit_xdma(0)
        xb_cur = emit_xbdma(0)
        att_prev = None
        for c in range(NCH + 1):
            att_cur = None
            if c < NCH:
                vg_sb = emit_proj(c)
                att_cur = emit_attention(c, vg_sb, xb_cur)
            if att_prev is not None:
                emit_o_outproj(c - 1, *att_prev)
            att_prev = att_cur
            if c + 1 < NCH:
                emit_xdma(c + 1)
                xb_cur = emit_xbdma(c + 1)

    nc.finalize()
    return nc


_NC_CACHE = {}


def get_nc(key="v3"):
    if key not in _NC_CACHE:
        _NC_CACHE[key] = build_program()
    return _NC_CACHE[key]


def make_in_maps(x, Wk, bk, Wq, bq, Wv, bv, Wo, bo):
    f = np.float32
    x = np.asarray(x, f)
    Wk, Wq, Wv, Wo = (np.asarray(a, f) for a in (Wk, Wq, Wv, Wo))
    bk, bq, bv, bo = (np.asarray(a, f) for a in (bk, bq, bv, bo))

    M = Wq.T @ Wk            # [C, C]: scores = x^T M x
    r = Wk.T @ bq            # [C]
    wvg = np.zeros((128, 128), f)
    wvg[64:128, 0:64] = M.T          # g rows: out[m,n] = sum_c M[m,c] x[c,n]
    wvg[64:128, 64] = r
    wvg[64:128, 96:128] = Wv.T
    woe = np.zeros((128, 64), f)
    woe[0:32, :] = Wo.T
    woe[32, :] = bo + Wo @ bv
    woe[64:128, :] = np.eye(64, dtype=f)
    idm = np.tile(np.eye(32, dtype=f), (4, 1))
    oz = np.zeros((32, W * D), f)
    oz[0, :] = 1.0

    consts = {"wvg": wvg, "woe": woe, "idm": idm, "oz": oz}
    in_maps = []
    for i in range(NCORES):
        xs = x[:, :, :, i * HS:(i + 1) * HS, :]          # [B, C, D, HS, W]
        xs = np.ascontiguousarray(np.transpose(xs, (0, 3, 1, 4, 2)))
        xs = xs.reshape(B, HS, C, W * D)
        m = {"x": xs, "xb": xs.astype(ml_dtypes.bfloat16)}
        m.update(consts)
        in_maps.append(m)
    return in_maps


def gather(results):
    out = np.empty((B, C, D, H, W), dtype=np.float32)
    for i in range(NCORES):
        yc = np.asarray(results[i]["y"]).astype(np.float32)
        yc = yc.reshape(B, HS, C, W, D)
        out[:, :, :, i * HS:(i + 1) * HS, :] = np.transpose(yc, (0, 2, 4, 1, 3))
    return out


def kernel(x, Wk, bk, Wq, bq, Wv, bv, Wo, bo):
    nc = get_nc()
    in_maps = make_in_maps(x, Wk, bk, Wq, bq, Wv, bv, Wo, bo)
    res = run_bass_kernel_spmd(nc, in_maps, core_ids=list(range(NCORES)))
    return gather(res.results)
